# revision 16
# baseline (speedup 1.0000x reference)
"""Trainium2 Bass kernel for nn_ContractiveMessageBlock (gnn_message_passing).

Strategy:
  - Shard edges across 8 cores by segment range: core c owns segments
    [250c, 250(c+1)).  Every edge goes to the core that owns its target
    segment, so the scatter-mean needs NO cross-core reduction.
  - Within a core, edges are sorted by local segment id; the scatter is a
    one-hot matmul (onehot[e, seg].T @ values[e, :]) accumulated in PSUM
    across the whole kernel.  Sorting means each 128-edge subtile usually
    touches a single 128-row segment block, halving scatter matmul work.
  - The radial basis (sin/env/unit) is tiny O(E*20) host math and is shipped
    as an augmented [21, E] operand so w_s = rbf_aug.T @ Wr_aug exactly
    (env premultiplied, br folded through the env row).
  - All matmul operands are bf16 (full PE rate, half DMA traffic);
    accumulation is fp32 in PSUM.  Counts + the final divide happen on host.
"""

import os
from contextlib import ExitStack

import numpy as np
import ml_dtypes

import concourse.bass as bass
import concourse.mybir as mybir
import concourse.tile as tile
from concourse.bass_utils import run_bass_kernel_spmd

BF16 = ml_dtypes.bfloat16

FEAT = 128
N_RBF = 20
CUTOFF = 5.0
N_CORES = 8
SEG_PER_CORE = 250
LOCAL_SEGS = 256          # 2 blocks of 128 PSUM rows; 250 real + pad rows
PAD_SEG = 255             # local segment id for padding edges
SUB = 128                 # edges per subtile (matmul contraction)
E_TILE = 512              # edges per outer tile
F3 = 3 * FEAT

LAST_EXEC_NS = None
LAST_RESULTS = None
LAST_NC = None
LAST_IN_MAPS = None

# Silu exists on HW ACT tables but not in CoreSim; sim_test flips this to
# False to validate structure with Sigmoid + explicit multiply instead.
ACT_SILU = True


def _host_prepare(s_i, v_i, r_iI, W1, b1, W2, b2, Wr, br, mapping):
    """Shard + sort edges per core, precompute rbf/env/unit, pack arrays."""
    E = s_i.shape[0]
    mapping = np.asarray(mapping).astype(np.int64).reshape(E)
    core_of = mapping // SEG_PER_CORE

    idx_per_core = []
    lm_per_core = []
    counts = []
    for c in range(N_CORES):
        idx = np.nonzero(core_of == c)[0]
        lm = (mapping[idx] - c * SEG_PER_CORE).astype(np.int32)
        order = np.argsort(lm, kind="stable")
        idx_per_core.append(idx[order])
        lm_per_core.append(lm[order])
        counts.append(idx.shape[0])

    P_pad = max(1, max(counts))
    P_pad = ((P_pad + E_TILE - 1) // E_TILE) * E_TILE
    n_sub = P_pad // SUB

    in_maps = []
    touched = [set() for _ in range(n_sub)]
    for c in range(N_CORES):
        idx = idx_per_core[c]
        lm = lm_per_core[c]
        n = idx.shape[0]

        sT = np.zeros((FEAT, P_pad), dtype=BF16)
        sT[:, :n] = np.asarray(s_i[idx], dtype=np.float32).T.astype(BF16)

        vkf = np.zeros((P_pad, F3), dtype=BF16)
        vkf[:n] = (
            np.asarray(v_i[idx], dtype=np.float32)
            .transpose(0, 2, 1)
            .reshape(n, F3)
            .astype(BF16)
        )

        r = np.asarray(r_iI[idx], dtype=np.float32)
        dist = np.sqrt((r * r + np.float32(1e-8)).sum(axis=-1))  # [n]
        unit = r / dist[:, None]
        nn = np.arange(1, N_RBF + 1, dtype=np.float32)
        rbf = np.sin(nn[None, :] * np.float32(np.pi) * dist[:, None] / CUTOFF) / dist[:, None]
        env = np.where(
            dist < CUTOFF,
            np.float32(0.5) * (np.cos(np.float32(np.pi) * dist / CUTOFF) + np.float32(1.0)),
            np.float32(0.0),
        )
        rbfT = np.zeros((N_RBF + 1, P_pad), dtype=BF16)
        rbfT[:N_RBF, :n] = (rbf * env[:, None]).T.astype(BF16)
        rbfT[N_RBF, :n] = env.astype(BF16)

        # per-edge scalars: unit_x, unit_y, unit_z, local segment id
        A = np.zeros((P_pad, 4), dtype=np.float32)
        A[:n, 0:3] = unit
        A[:, 3] = PAD_SEG
        A[:n, 3] = lm.astype(np.float32)
        umap = A.reshape(n_sub, SUB, 4).transpose(1, 0, 2).reshape(SUB, n_sub * 4)
        umap = np.ascontiguousarray(umap)

        for s in range(n_sub):
            lo = s * SUB
            hi = min(lo + SUB, n)
            if hi > lo:
                touched[s].add(int(lm[lo]) // 128)
                touched[s].add(int(lm[hi - 1]) // 128)

        in_maps.append(dict(sT=sT, vkf=vkf, rbfT=rbfT, umap=umap))

    # shared (replicated) weights
    W1b = np.asarray(W1, dtype=np.float32).astype(BF16)
    W2b = np.asarray(W2, dtype=np.float32).astype(BF16)
    WrP = np.zeros((N_RBF + 1, F3), dtype=BF16)
    WrP[:N_RBF] = np.asarray(Wr, dtype=np.float32).astype(BF16)
    WrP[N_RBF] = np.asarray(br, dtype=np.float32).astype(BF16)
    b1c = np.asarray(b1, dtype=np.float32).reshape(FEAT, 1).copy()
    iota = np.broadcast_to(
        np.arange(LOCAL_SEGS, dtype=np.float32), (SUB, LOCAL_SEGS)
    ).astype(BF16)

    b2a = np.asarray(b2, dtype=np.float32).reshape(F3)
    have_b2 = bool(np.any(b2a))
    b2rep = np.broadcast_to(b2a, (SUB, F3)).astype(np.float32).copy() if have_b2 else None

    for m in in_maps:
        m.update(W1=W1b, W2=W2b, WrP=WrP, b1=b1c, iota=iota)
        if have_b2:
            m.update(b2rep=b2rep)

    return in_maps, lm_per_core, P_pad, touched, have_b2


def _build_bass(P_pad, touched, have_b2, reps=1):
    dt = mybir.dt
    n_sub = P_pad // SUB
    ntiles = P_pad // E_TILE

    nc = bass.Bass()
    sT = nc.declare_dram_parameter("sT", [FEAT, P_pad], dt.bfloat16, False)
    vkf = nc.declare_dram_parameter("vkf", [P_pad, F3], dt.bfloat16, False)
    rbfT = nc.declare_dram_parameter("rbfT", [N_RBF + 1, P_pad], dt.bfloat16, False)
    umap = nc.declare_dram_parameter("umap", [SUB, n_sub * 4], dt.float32, False)
    W1 = nc.declare_dram_parameter("W1", [FEAT, FEAT], dt.bfloat16, False)
    W2 = nc.declare_dram_parameter("W2", [FEAT, F3], dt.bfloat16, False)
    WrP = nc.declare_dram_parameter("WrP", [N_RBF + 1, F3], dt.bfloat16, False)
    b1 = nc.declare_dram_parameter("b1", [FEAT, 1], dt.float32, False)
    iota = nc.declare_dram_parameter("iota", [SUB, LOCAL_SEGS], dt.bfloat16, False)
    if have_b2:
        b2rep = nc.declare_dram_parameter("b2rep", [SUB, F3], dt.float32, False)
    out_a = nc.declare_dram_parameter("out_a", [LOCAL_SEGS, 512], dt.float32, True)
    out_b = nc.declare_dram_parameter("out_b", [LOCAL_SEGS, F3], dt.float32, True)

    first = {}
    last = {}
    for s in range(n_sub):
        for b in touched[s]:
            if b not in first:
                first[b] = s
            last[b] = s

    with ExitStack() as ctx:
        tc = ctx.enter_context(tile.TileContext(nc))
        cpool = ctx.enter_context(tc.tile_pool(name="const", bufs=1))
        spool = ctx.enter_context(tc.tile_pool(name="sb", bufs=2))
        vpool = ctx.enter_context(tc.tile_pool(name="vv", bufs=3))
        ppool = ctx.enter_context(tc.tile_pool(name="ps", bufs=1, space="PSUM"))
        pmpool = ctx.enter_context(tc.tile_pool(name="pm", bufs=2, space="PSUM"))

        W1t = cpool.tile([FEAT, FEAT], dt.bfloat16, tag="W1")
        nc.sync.dma_start(out=W1t[:], in_=W1[:, :])
        W2t = cpool.tile([FEAT, F3], dt.bfloat16, tag="W2")
        nc.sync.dma_start(out=W2t[:], in_=W2[:, :])
        Wrt = cpool.tile([N_RBF + 1, F3], dt.bfloat16, tag="Wr")
        nc.sync.dma_start(out=Wrt[:], in_=WrP[:, :])
        b1t = cpool.tile([FEAT, 1], dt.float32, tag="b1")
        nc.sync.dma_start(out=b1t[:], in_=b1[:, :])
        iot = cpool.tile([SUB, LOCAL_SEGS], dt.bfloat16, tag="iota")
        nc.sync.dma_start(out=iot[:], in_=iota[:, :])
        if have_b2:
            b2t = cpool.tile([SUB, F3], dt.float32, tag="b2")
            nc.sync.dma_start(out=b2t[:], in_=b2rep[:, :])

        # whole umap is tiny (n_sub*4 fp32 cols); load once
        um_all = cpool.tile([SUB, n_sub * 4], dt.float32, tag="um_all")
        nc.sync.dma_start(out=um_all[:], in_=umap[:, :])

        # Warm ACT's view of the b1 DMA so the per-tile Silu carries only
        # one sync wait (the ACT instruction struct allows a single wait).
        b1warm = cpool.tile([FEAT, 1], dt.float32, tag="b1warm")
        nc.scalar.activation(
            out=b1warm[:], in_=b1t[:], func=mybir.ActivationFunctionType.Copy
        )

        acc1 = [
            ppool.tile([128, 512], dt.float32, tag=f"acc1_{b}", name=f"acc1_{b}")
            for b in (0, 1)
        ]
        acc2 = [
            ppool.tile([128, F3], dt.float32, tag=f"acc2_{b}", name=f"acc2_{b}")
            for b in (0, 1)
        ]

        for rep in range(reps):
          for t in range(ntiles):
            e0 = t * E_TILE
            sT_t = spool.tile([FEAT, E_TILE], dt.bfloat16, tag="sT")
            nc.sync.dma_start(out=sT_t[:], in_=sT[:, e0 : e0 + E_TILE])
            rbf_t = spool.tile([N_RBF + 1, E_TILE], dt.bfloat16, tag="rbf")
            nc.sync.dma_start(out=rbf_t[:], in_=rbfT[:, e0 : e0 + E_TILE])

            phi1_ps = ppool.tile([FEAT, E_TILE], dt.float32, tag="phi1")
            nc.tensor.matmul(out=phi1_ps[:], lhsT=W1t[:], rhs=sT_t[:], start=True, stop=True)
            phi1s = spool.tile([FEAT, E_TILE], dt.bfloat16, tag="phi1s")
            if ACT_SILU:
                nc.scalar.activation(
                    out=phi1s[:], in_=phi1_ps[:],
                    func=mybir.ActivationFunctionType.Silu,
                    bias=b1t[:, 0:1], scale=1.0,
                )
            else:
                # silu(x+b1) = (x+b1) * sigmoid(x+b1); two ACT passes + DVE mul
                sg = spool.tile([FEAT, E_TILE], dt.bfloat16, tag="sg")
                nc.scalar.activation(
                    out=sg[:], in_=phi1_ps[:],
                    func=mybir.ActivationFunctionType.Sigmoid,
                    bias=b1t[:, 0:1], scale=1.0,
                )
                xb = spool.tile([FEAT, E_TILE], dt.bfloat16, tag="xb")
                nc.scalar.activation(
                    out=xb[:], in_=phi1_ps[:],
                    func=mybir.ActivationFunctionType.Identity,
                    bias=b1t[:, 0:1], scale=1.0,
                )
                nc.vector.tensor_tensor(
                    out=phi1s[:], in0=xb[:], in1=sg[:], op=mybir.AluOpType.mult
                )

            for j in range(4):
                s = t * 4 + j
                f0 = j * SUB
                v_t = vpool.tile([SUB, F3], dt.bfloat16, tag="v")
                nc.sync.dma_start(out=v_t[:], in_=vkf[e0 + f0 : e0 + f0 + SUB, :])

                phi_ps = pmpool.tile([SUB, F3], dt.float32, tag="phim")
                nc.tensor.matmul(
                    out=phi_ps[:], lhsT=phi1s[:, f0 : f0 + SUB], rhs=W2t[:],
                    start=True, stop=True,
                )
                ws_ps = ppool.tile([SUB, F3], dt.float32, tag="wsm")
                nc.tensor.matmul(
                    out=ws_ps[:], lhsT=rbf_t[:, f0 : f0 + SUB], rhs=Wrt[:],
                    start=True, stop=True,
                )

                # DVE cannot read two PSUM operands in one op: evacuate ws
                # to SBUF first (on DVE, keeping instruction waits <= 1 for
                # the ACT engine whose ISA slot allows a single sync wait).
                ws_sb = spool.tile([SUB, F3], dt.bfloat16, tag="ws_sb")
                nc.vector.tensor_copy(ws_sb[:], ws_ps[:])
                inv = spool.tile([SUB, F3], dt.bfloat16, tag="inv")
                if have_b2:
                    phib = spool.tile([SUB, F3], dt.float32, tag="phib")
                    nc.vector.tensor_tensor(
                        out=phib[:], in0=phi_ps[:], in1=b2t[:], op=mybir.AluOpType.add
                    )
                    nc.vector.tensor_tensor(
                        out=inv[:], in0=phib[:], in1=ws_sb[:], op=mybir.AluOpType.mult
                    )
                else:
                    nc.vector.tensor_tensor(
                        out=inv[:], in0=phi_ps[:], in1=ws_sb[:], op=mybir.AluOpType.mult
                    )

                # val = [split_1 | split_0 * v] so the s-part and sv-part land
                # in ONE matmul / ONE PSUM accumulation group per bank
                val = spool.tile([SUB, 512], dt.bfloat16, tag="val")
                nc.vector.tensor_copy(val[:, 0:FEAT], inv[:, FEAT : 2 * FEAT])
                for k in range(3):
                    nc.vector.tensor_tensor(
                        out=val[:, FEAT + k * FEAT : FEAT + (k + 1) * FEAT],
                        in0=v_t[:, k * FEAT : (k + 1) * FEAT],
                        in1=inv[:, 0:FEAT],
                        op=mybir.AluOpType.mult,
                    )
                ut = spool.tile([SUB, F3], dt.bfloat16, tag="ut")
                for k in range(3):
                    nc.gpsimd.tensor_scalar(
                        ut[:, k * FEAT : (k + 1) * FEAT],
                        inv[:, 2 * FEAT : 3 * FEAT],
                        um_all[:, t * 16 + 4 * j + k : t * 16 + 4 * j + k + 1],
                        None,
                        mybir.AluOpType.mult,
                    )
                oh = spool.tile([SUB, LOCAL_SEGS], dt.bfloat16, tag="oh")
                nc.gpsimd.tensor_scalar(
                    oh[:], iot[:],
                    um_all[:, t * 16 + 4 * j + 3 : t * 16 + 4 * j + 4],
                    None,
                    mybir.AluOpType.is_equal,
                )

                for b in sorted(touched[s]):
                    st = first[b] == s
                    sp = last[b] == s
                    ohb = oh[:, b * 128 : (b + 1) * 128]
                    nc.tensor.matmul(
                        out=acc1[b][:, :], lhsT=ohb, rhs=val[:],
                        start=st, stop=sp, skip_group_check=True,
                    )
                    nc.tensor.matmul(
                        out=acc2[b][:, :], lhsT=ohb, rhs=ut[:],
                        start=st, stop=sp, skip_group_check=True,
                    )

        for b in (0, 1):
            ea = spool.tile([128, 512], dt.float32, tag="ev_a")
            eb = spool.tile([128, F3], dt.float32, tag="ev_b")
            if b in first:
                nc.vector.tensor_copy(ea[:], acc1[b][:])
                nc.vector.tensor_copy(eb[:], acc2[b][:])
            else:
                nc.vector.memset(ea[:], 0.0)
                nc.vector.memset(eb[:], 0.0)
            nc.sync.dma_start(out=out_a[b * 128 : (b + 1) * 128, :], in_=ea[:])
            nc.sync.dma_start(out=out_b[b * 128 : (b + 1) * 128, :], in_=eb[:])

    return nc


def _legalize_waits(nc):
    """This walrus build allows one attached sync-wait per instruction
    (two for EventSemaphore); Tile attaches more.  Hoist extras onto NoOp
    instructions inserted immediately before, on the same engine queue —
    semantics identical (same waits, same queue order)."""
    import orjson

    d = orjson.loads(nc.to_json_bytes())
    n_split = 0
    w_id = 0
    for fn in d["functions"]:
        for blk in fn["blocks"]:
            new_list = []
            for ins in blk["instructions"]:
                si = ins.get("sync_info")
                waits = (si or {}).get("on_wait") or []
                cap = 2 if ins.get("opcode") == "EventSemaphore" else 1
                if len(waits) > cap:
                    n_split += 1
                    for w in waits[:-cap]:
                        w_id += 1
                        new_list.append({
                            "debug": ins.get("debug", 0),
                            "engine": ins["engine"],
                            "ins": [],
                            "outs": [],
                            "name": f"I-WX{w_id}",
                            "opcode": "NoOp",
                            "sync_info": {"on_update": [], "on_wait": [w]},
                        })
                    si["on_wait"] = waits[-cap:]
                new_list.append(ins)
            blk["instructions"] = new_list
    nc.m = mybir.module_from_json_bytes(orjson.dumps(d))
    return n_split


def kernel(s_i, v_i, r_iI, W1, b1, W2, b2, Wr, br, mapping, num_cg):
    global LAST_EXEC_NS, LAST_RESULTS
    num_cg = int(np.asarray(num_cg))
    assert num_cg == N_CORES * SEG_PER_CORE, num_cg
    E = np.asarray(s_i).shape[0]

    in_maps, lm_per_core, P_pad, touched, have_b2 = _host_prepare(
        s_i, v_i, r_iI, W1, b1, W2, b2, Wr, br, mapping
    )
    nc = _build_bass(P_pad, touched, have_b2)
    _legalize_waits(nc)
    global LAST_NC, LAST_IN_MAPS
    LAST_NC = nc
    LAST_IN_MAPS = in_maps
    res = run_bass_kernel_spmd(nc, in_maps, list(range(N_CORES)))
    LAST_EXEC_NS = res.exec_time_ns
    LAST_RESULTS = res

    mapping_np = np.asarray(mapping).astype(np.int64).reshape(E)
    cnt = np.bincount(mapping_np, minlength=num_cg).astype(np.float32)
    cnt_c = np.maximum(cnt, 1.0)

    delta_s = np.zeros((num_cg, FEAT), dtype=np.float32)
    delta_v = np.zeros((num_cg, FEAT, 3), dtype=np.float32)
    for c in range(N_CORES):
        out_a = np.array(res.results[c]["out_a"], dtype=np.float32)
        out_b = np.array(res.results[c]["out_b"], dtype=np.float32)
        lo = c * SEG_PER_CORE
        hi = lo + SEG_PER_CORE
        s_sum = out_a[:SEG_PER_CORE, 0:FEAT]
        v_sum = (
            out_a[:SEG_PER_CORE, FEAT:512].reshape(SEG_PER_CORE, 3, FEAT)
            + out_b[:SEG_PER_CORE].reshape(SEG_PER_CORE, 3, FEAT)
        )
        empty = cnt[lo:hi] == 0.0
        s_sum[empty] = 0.0
        v_sum[empty] = 0.0
        delta_s[lo:hi] = s_sum / cnt_c[lo:hi, None]
        delta_v[lo:hi] = v_sum.transpose(0, 2, 1) / cnt_c[lo:hi, None, None]

    return (delta_s, delta_v)
